# revision 7
# baseline (speedup 1.0000x reference)
"""Trainium2 Bass kernel for nn_DeepQNetwork (conv encoder + 8-expert MLP head).

Strategy: data-parallel over 8 NeuronCores (256 samples each). Convs are
mapped to TensorE matmuls via host-side space-to-depth (stride-s conv ->
s*s-folded channels, kernel split into 2x2 accumulation steps). The expert
MLP computes all 8 experts (expert-paired on the M dim, block-diagonal
weights for the 64x64 layers); the per-sample expert row is gathered on the
host. bf16 operands with fp32 PSUM accumulation.

Self-contained: only concourse/numpy imports, shapes hardcoded.
"""

import os
import sys

import ml_dtypes
import numpy as np

BF16 = ml_dtypes.bfloat16

B, E, A = 2048, 8, 6
NCORES = 8
BC = B // NCORES  # 256 samples per core
CHUNK = 32  # conv1/conv2 batch chunk (16 per half)
HALF = 16


def _install_axon_prof_shim():
    """Register the NTFF profile hook (exec-time measurement) under axon."""
    import sys
    import types

    if "antenv.axon_hooks" not in sys.modules:
        mod = types.ModuleType("antenv.axon_hooks")
        _hook = [None]
        mod.set_axon_ntff_profile_hook = lambda h: _hook.__setitem__(0, h)
        mod.get_axon_ntff_profile_hook = lambda: _hook[0]
        sys.modules["antenv.axon_hooks"] = mod
        import antenv

        antenv.axon_hooks = mod
    from antenv.axon_hooks import (
        get_axon_ntff_profile_hook,
        set_axon_ntff_profile_hook,
    )

    if get_axon_ntff_profile_hook() is None:
        try:
            from trn_agent_boot.trn_boot import _ntff_profile_via_ctypes

            set_axon_ntff_profile_hook(
                _ntff_profile_via_ctypes("/opt/axon/libaxon_pjrt.so")
            )
        except Exception:
            pass
    import concourse.bass_utils as bu

    bu.upload_artifacts = lambda tmpdir: tmpdir


def _build_program():
    import concourse.mybir as mybir
    import concourse.tile as tile
    from concourse import bacc

    dt = mybir.dt
    AF = mybir.ActivationFunctionType
    ALU = mybir.AluOpType

    nc = bacc.Bacc(
        "TRN2", target_bir_lowering=False, debug=False, num_devices=NCORES
    )

    # ---- DRAM tensors ----
    x1a_d = nc.dram_tensor("x1a", [48, 128, 21, 21], dt.bfloat16, kind="ExternalInput")
    x1b_d = nc.dram_tensor("x1b", [48, 128, 21, 21], dt.bfloat16, kind="ExternalInput")
    k1_d = nc.dram_tensor("k1r", [48, 128], dt.bfloat16, kind="ExternalInput")
    k2_d = nc.dram_tensor("k2r", [128, 256], dt.bfloat16, kind="ExternalInput")
    k3_d = nc.dram_tensor("k3r", [64, 576], dt.bfloat16, kind="ExternalInput")
    w1_d = nc.dram_tensor("w1r", [4, 49, 64, 128], dt.bfloat16, kind="ExternalInput")
    w25_d = nc.dram_tensor("w25r", [128, 16 * 128], dt.bfloat16, kind="ExternalInput")
    w6_d = nc.dram_tensor("w6r", [128, 48], dt.bfloat16, kind="ExternalInput")
    c1_d = nc.dram_tensor("c1t", [128, 1], dt.float32, kind="ExternalInput")
    c2_d = nc.dram_tensor("c2t", [64, 1], dt.float32, kind="ExternalInput")
    c3_d = nc.dram_tensor("c3t", [64, 1], dt.float32, kind="ExternalInput")
    bm_d = nc.dram_tensor("bmlp", [128, 20], dt.float32, kind="ExternalInput")
    b6_d = nc.dram_tensor("b6t", [12, 4], dt.float32, kind="ExternalInput")
    out_d = nc.dram_tensor("out", [12, 4, 256], dt.float32, kind="ExternalOutput")
    debug = bool(os.environ.get("NN_KERNEL_DEBUG"))
    if debug:
        dbgx2_d = nc.dram_tensor("dbg_x2", [128, 32, 10, 10], dt.bfloat16, kind="ExternalOutput")
        dbgx3_d = nc.dram_tensor("dbg_x3", [64, 256, 9, 9], dt.bfloat16, kind="ExternalOutput")
        dbgx4_d = nc.dram_tensor("dbg_x4", [64, 256, 7, 7], dt.bfloat16, kind="ExternalOutput")
        dbgh1_d = nc.dram_tensor("dbg_h1", [128, 4, 256], dt.bfloat16, kind="ExternalOutput")
        dbgh5_d = nc.dram_tensor("dbg_h5", [128, 4, 256], dt.bfloat16, kind="ExternalOutput")

    with tile.TileContext(nc) as tc:
        with (
            tc.tile_pool(name="wts", bufs=1) as wpool,
            tc.tile_pool(name="x1", bufs=2) as x1pool,
            tc.tile_pool(name="x2", bufs=2) as x2pool,
            tc.tile_pool(name="big", bufs=1) as bigpool,
            tc.tile_pool(name="w1s", bufs=8) as w1pool,
            tc.tile_pool(name="psc", bufs=4, space="PSUM") as psc,
            tc.tile_pool(name="psm", bufs=2, space="PSUM") as psm,
        ):
            # ---- resident weights/biases ----
            k1sb = wpool.tile([128, 128], dt.bfloat16)  # two copies: rows 0-47, 64-111
            nc.sync.dma_start(k1sb[0:48], k1_d.ap())
            nc.sync.dma_start(k1sb[64:112], k1_d.ap())
            k2sb = wpool.tile([128, 256], dt.bfloat16)
            nc.sync.dma_start(k2sb[:], k2_d.ap())
            k3sb = wpool.tile([64, 576], dt.bfloat16)
            nc.sync.dma_start(k3sb[:], k3_d.ap())
            w25sb = wpool.tile([128, 16 * 128], dt.bfloat16)
            nc.sync.dma_start(w25sb[:], w25_d.ap())
            w6sb = wpool.tile([128, 48], dt.bfloat16)
            nc.sync.dma_start(w6sb[:], w6_d.ap())
            c1sb = wpool.tile([128, 1], dt.float32)
            nc.sync.dma_start(c1sb[:], c1_d.ap())
            c2sb = wpool.tile([64, 1], dt.float32)
            nc.sync.dma_start(c2sb[:], c2_d.ap())
            c3sb = wpool.tile([64, 1], dt.float32)
            nc.sync.dma_start(c3sb[:], c3_d.ap())
            bmsb = wpool.tile([128, 20], dt.float32)
            nc.sync.dma_start(bmsb[:], bm_d.ap())
            b6sb = wpool.tile([12, 4], dt.float32)
            nc.sync.dma_start(b6sb[:], b6_d.ap())

            X3t = bigpool.tile([64, 256, 9, 9], dt.bfloat16)
            X4t = bigpool.tile([64, 256, 7, 7], dt.bfloat16)
            Ha = bigpool.tile([128, 4, 256], dt.bfloat16)
            Hb = bigpool.tile([128, 4, 256], dt.bfloat16)
            OUTt = bigpool.tile([12, 4, 256], dt.float32)

            copy_ctr = 0

            def relu_copy(dst, src, bias):
                nonlocal copy_ctr
                copy_ctr += 1
                if copy_ctr % 2 == 0:
                    nc.scalar.activation(dst, src, AF.Relu, bias=bias)
                else:
                    nc.vector.tensor_scalar(dst, src, bias, 0.0, ALU.add, ALU.max)

            # ---- conv1 + conv2, chunked over batch ----
            for c in range(8):
                X1t = x1pool.tile([128, HALF, 21, 21], dt.bfloat16)
                nc.sync.dma_start(
                    X1t[0:48], x1a_d.ap()[:, c * HALF : (c + 1) * HALF]
                )
                nc.sync.dma_start(
                    X1t[64:112], x1b_d.ap()[:, c * HALF : (c + 1) * HALF]
                )
                X2t = x2pool.tile([128, CHUNK, 10, 10], dt.bfloat16)

                # conv1: K=48 x 4 accumulation steps, 4 col-tiles (output parity)
                for g8 in range(8):  # alternate halves for LDW overlap
                    h, g = g8 % 2, g8 // 2
                    base = 64 * h
                    X1h = X1t[base : base + 48]
                    pt = psc.tile([128, 4, 10, 10], dt.float32, tag="conv")
                    for s in range(4):
                        a, a2 = s >> 1, s & 1
                        for q in range(4):
                            di, dj = q >> 1, q & 1
                            rhs = X1h[
                                :,
                                4 * g : 4 * g + 4,
                                (di + a) : (di + a + 19) : 2,
                                (dj + a2) : (dj + a2 + 19) : 2,
                            ]
                            nc.tensor.matmul(
                                pt[32 * q : 32 * q + 32],
                                k1sb[base : base + 48, 32 * s : 32 * s + 32],
                                rhs,
                                start=(s == 0),
                                stop=(s == 3),
                                tile_position=(base, 32 * q),
                            )
                    relu_copy(X2t[:, HALF * h + 4 * g : HALF * h + 4 * g + 4], pt[:], c1sb[:])

                if debug and c == 0:
                    nc.sync.dma_start(dbgx2_d.ap(), X2t[:])

                # conv2: K=128 x 4 steps
                for g in range(8):
                    pt2 = psc.tile([64, 4, 9, 9], dt.float32, tag="conv")
                    for s in range(4):
                        a, a2 = s >> 1, s & 1
                        rhs = X2t[:, 4 * g : 4 * g + 4, a : a + 9, a2 : a2 + 9]
                        nc.tensor.matmul(
                            pt2[:],
                            k2sb[:, 64 * s : 64 * s + 64],
                            rhs,
                            start=(s == 0),
                            stop=(s == 3),
                        )
                    b0 = c * CHUNK + 4 * g
                    relu_copy(X3t[:, b0 : b0 + 4], pt2[:], c2sb[:])

            # ---- conv3: K=64 x 9 steps ----
            for g in range(32):
                pt3 = psc.tile([64, 8, 7, 7], dt.float32, tag="conv")
                for k in range(9):
                    ky, kx = k // 3, k % 3
                    rhs = X3t[:, 8 * g : 8 * g + 8, ky : ky + 7, kx : kx + 7]
                    nc.tensor.matmul(
                        pt3[:],
                        k3sb[:, 64 * k : 64 * k + 64],
                        rhs,
                        start=(k == 0),
                        stop=(k == 8),
                    )
                relu_copy(X4t[:, 8 * g : 8 * g + 8], pt3[:], c3sb[:])

            if debug:
                nc.sync.dma_start(dbgx3_d.ap(), X3t[:])
                nc.sync.dma_start(dbgx4_d.ap(), X4t[:])

            # ---- MLP layer 1: all 8 experts, expert-paired M=128 ----
            for p in range(4):
                ptm = psm.tile([128, 256], dt.float32, tag="mlp")
                for s in range(49):
                    sy, sx = s // 7, s % 7
                    wt = w1pool.tile([64, 128], dt.bfloat16)
                    nc.sync.dma_start(wt[:], w1_d.ap()[p, s])
                    nc.tensor.matmul(
                        ptm[:],
                        wt[:],
                        X4t[:, :, sy, sx],
                        start=(s == 0),
                        stop=(s == 48),
                    )
                relu_copy(Ha[:, p, :], ptm[:], bmsb[:, p : p + 1])

            if debug:
                nc.sync.dma_start(dbgh1_d.ap(), Ha[:])

            # ---- MLP layers 2-5: block-diagonal expert pairs ----
            src, dst = Ha, Hb
            for l in range(4):
                for p in range(4):
                    idx = l * 4 + p
                    ptm = psm.tile([128, 256], dt.float32, tag="mlp")
                    nc.tensor.matmul(
                        ptm[:],
                        w25sb[:, 128 * idx : 128 * idx + 128],
                        src[:, p, :],
                        start=True,
                        stop=True,
                    )
                    relu_copy(dst[:, p, :], ptm[:], bmsb[:, (l + 1) * 4 + p : (l + 1) * 4 + p + 1])
                src, dst = dst, src

            if debug:
                nc.sync.dma_start(dbgh5_d.ap(), src[:])

            # ---- MLP layer 6 (no relu) ----
            for p in range(4):
                pt6 = psm.tile([12, 256], dt.float32, tag="mlp")
                nc.tensor.matmul(
                    pt6[:],
                    w6sb[:, 12 * p : 12 * p + 12],
                    src[:, p, :],
                    start=True,
                    stop=True,
                )
                nc.vector.tensor_scalar(
                    OUTt[:, p, :], pt6[:], b6sb[:, p : p + 1], None, ALU.add
                )

            nc.sync.dma_start(out_d.ap(), OUTt[:])

    nc.compile()
    return nc


_prog_cache = {}
LAST_RESULTS = None


def kernel(**inputs):
    state = np.asarray(inputs["state"], dtype=np.float32)
    rm_state = np.asarray(inputs["rm_state"]).astype(np.int64)
    k1 = np.asarray(inputs["k1"], dtype=np.float32)
    c1 = np.asarray(inputs["c1"], dtype=np.float32)
    k2 = np.asarray(inputs["k2"], dtype=np.float32)
    c2 = np.asarray(inputs["c2"], dtype=np.float32)
    k3 = np.asarray(inputs["k3"], dtype=np.float32)
    c3 = np.asarray(inputs["c3"], dtype=np.float32)
    Ws = [np.asarray(inputs[f"W{i}"], dtype=np.float32) for i in range(1, 7)]
    Bs = [np.asarray(inputs[f"B{i}"], dtype=np.float32) for i in range(1, 7)]

    # ---- host prep ----
    # space-to-depth: [core, b, (i,dy), (j,dx), c] -> [core, (dy,dx,c), b, i, j]
    v = state.reshape(NCORES, BC, 21, 4, 21, 4, 3)
    s2d = np.ascontiguousarray(np.transpose(v, (0, 3, 5, 6, 1, 2, 4))).reshape(
        NCORES, 48, BC, 21, 21
    )
    # chunk c processes samples [32c, 32c+32): first 16 via x1a, next 16 via
    # x1b (the two SBUF partition-halves), so device order == global order.
    vv = s2d.reshape(NCORES, 48, 8, 2, 16, 21, 21)
    x1a = np.ascontiguousarray(vv[:, :, :, 0]).reshape(NCORES, 48, 128, 21, 21).astype(BF16)
    x1b = np.ascontiguousarray(vv[:, :, :, 1]).reshape(NCORES, 48, 128, 21, 21).astype(BF16)

    # conv weights: partition=(dy,dx,ci), free=(step, co)
    k1r = (
        k1.reshape(32, 3, 2, 4, 2, 4)
        .transpose(2, 4, 3, 5, 1, 0)
        .reshape(4, 48, 32)
        .transpose(1, 0, 2)
        .reshape(48, 128)
        .astype(BF16)
    )
    k2r = (
        k2.reshape(64, 32, 2, 2, 2, 2)
        .transpose(2, 4, 3, 5, 1, 0)
        .reshape(4, 128, 64)
        .transpose(1, 0, 2)
        .reshape(128, 256)
        .astype(BF16)
    )
    k3r = (
        k3.transpose(2, 3, 1, 0).reshape(9, 64, 64).transpose(1, 0, 2).reshape(64, 576)
    ).astype(BF16)

    # W1: [E, 3136, 64] -> [pair, s, c, (e2 m)]
    w1r = np.ascontiguousarray(
        Ws[0].reshape(4, 2, 64, 49, 64).transpose(0, 3, 2, 1, 4)
    ).reshape(4, 49, 64, 128).astype(BF16)

    # W2..W5 block-diagonal expert pairs: [128, 16*128] (col = (l*4+p)*128 + m')
    w25 = np.zeros((128, 16, 128), np.float32)
    for l in range(4):
        Wl = Ws[1 + l]
        for p in range(4):
            for e2 in range(2):
                w25[64 * e2 : 64 * e2 + 64, l * 4 + p, 64 * e2 : 64 * e2 + 64] = Wl[
                    2 * p + e2
                ]
    w25 = w25.reshape(128, 16 * 128).astype(BF16)

    w6 = np.zeros((128, 4, 12), np.float32)
    for p in range(4):
        for e2 in range(2):
            w6[64 * e2 : 64 * e2 + 64, p, 6 * e2 : 6 * e2 + 6] = Ws[5][2 * p + e2]
    w6 = w6.reshape(128, 48).astype(BF16)

    c1t = np.tile(c1, 4)[:, None].astype(np.float32)
    c2t = c2[:, None].astype(np.float32)
    c3t = c3[:, None].astype(np.float32)
    bmlp = np.zeros((128, 20), np.float32)
    for l in range(5):
        Bl = Bs[l]
        for p in range(4):
            bmlp[:, l * 4 + p] = np.concatenate([Bl[2 * p], Bl[2 * p + 1]])
    b6t = np.zeros((12, 4), np.float32)
    for p in range(4):
        b6t[:, p] = np.concatenate([Bs[5][2 * p], Bs[5][2 * p + 1]])

    # ---- build + run ----
    trace = bool(os.environ.get("NN_KERNEL_TRACE"))
    _install_axon_prof_shim()
    if "nc" not in _prog_cache:
        _prog_cache["nc"] = _build_program()
    nc = _prog_cache["nc"]

    shared = {
        "k1r": k1r,
        "k2r": k2r,
        "k3r": k3r,
        "w1r": w1r,
        "w25r": w25,
        "w6r": w6,
        "c1t": c1t,
        "c2t": c2t,
        "c3t": c3t,
        "bmlp": bmlp,
        "b6t": b6t,
    }
    in_maps = [
        {"x1a": np.ascontiguousarray(x1a[c]), "x1b": np.ascontiguousarray(x1b[c]), **shared}
        for c in range(NCORES)
    ]

    from concourse.bass_utils import run_bass_kernel_spmd

    res = run_bass_kernel_spmd(
        nc, in_maps, core_ids=list(range(NCORES)), trace=trace
    )
    if trace and res.exec_time_ns is not None:
        print(f"HW exec time: {res.exec_time_ns} ns")

    global LAST_RESULTS
    LAST_RESULTS = res.results

    # ---- host gather ----
    outs = []
    for c in range(NCORES):
        r = res.results[c]["out"].reshape(2, 6, 4, 256)
        outs.append(r.transpose(2, 0, 3, 1).reshape(8, 256, 6))
    full = np.stack(outs)  # [core, e, b, a]
    per_sample = full.transpose(0, 2, 1, 3).reshape(B, E, A)
    return per_sample[np.arange(B), rm_state].astype(np.float32)


# revision 8
# speedup vs baseline: 1.3240x; 1.3240x over previous
"""Trainium2 Bass kernel for nn_DeepQNetwork (conv encoder + 8-expert MLP head).

Strategy: data-parallel over 8 NeuronCores (256 samples each). Convs are
mapped to TensorE matmuls via host-side space-to-depth (stride-s conv ->
s*s-folded channels, kernel split into 2x2 accumulation steps). The expert
MLP computes all 8 experts (expert-paired on the M dim, block-diagonal
weights for the 64x64 layers); the per-sample expert row is gathered on the
host. bf16 operands with fp32 PSUM accumulation.

Self-contained: only concourse/numpy imports, shapes hardcoded.
"""

import os
import sys

import ml_dtypes
import numpy as np

BF16 = ml_dtypes.bfloat16

B, E, A = 2048, 8, 6
NCORES = 8
BC = B // NCORES  # 256 samples per core
CHUNK = 32  # conv1/conv2 batch chunk (16 per half)
HALF = 16


def _install_axon_prof_shim():
    """Register the NTFF profile hook (exec-time measurement) under axon."""
    import sys
    import types

    if "antenv.axon_hooks" not in sys.modules:
        mod = types.ModuleType("antenv.axon_hooks")
        _hook = [None]
        mod.set_axon_ntff_profile_hook = lambda h: _hook.__setitem__(0, h)
        mod.get_axon_ntff_profile_hook = lambda: _hook[0]
        sys.modules["antenv.axon_hooks"] = mod
        import antenv

        antenv.axon_hooks = mod
    from antenv.axon_hooks import (
        get_axon_ntff_profile_hook,
        set_axon_ntff_profile_hook,
    )

    if get_axon_ntff_profile_hook() is None:
        try:
            from trn_agent_boot.trn_boot import _ntff_profile_via_ctypes

            set_axon_ntff_profile_hook(
                _ntff_profile_via_ctypes("/opt/axon/libaxon_pjrt.so")
            )
        except Exception:
            pass
    import concourse.bass_utils as bu

    bu.upload_artifacts = lambda tmpdir: tmpdir


def _build_program():
    import concourse.mybir as mybir
    import concourse.tile as tile
    from concourse import bacc

    dt = mybir.dt
    AF = mybir.ActivationFunctionType
    ALU = mybir.AluOpType

    nc = bacc.Bacc(
        "TRN2", target_bir_lowering=False, debug=False, num_devices=NCORES
    )

    # ---- DRAM tensors ----
    x1a_d = nc.dram_tensor("x1a", [48, 128, 21, 21], dt.bfloat16, kind="ExternalInput")
    x1b_d = nc.dram_tensor("x1b", [48, 128, 21, 21], dt.bfloat16, kind="ExternalInput")
    k1_d = nc.dram_tensor("k1r", [48, 128], dt.bfloat16, kind="ExternalInput")
    k2_d = nc.dram_tensor("k2r", [128, 256], dt.bfloat16, kind="ExternalInput")
    k3_d = nc.dram_tensor("k3r", [64, 576], dt.bfloat16, kind="ExternalInput")
    w1_d = nc.dram_tensor("w1r", [128, 4 * 25 * 128], dt.bfloat16, kind="ExternalInput")
    w25_d = nc.dram_tensor("w25r", [128, 16 * 128], dt.bfloat16, kind="ExternalInput")
    w6_d = nc.dram_tensor("w6r", [128, 48], dt.bfloat16, kind="ExternalInput")
    c1_d = nc.dram_tensor("c1t", [128, 1], dt.float32, kind="ExternalInput")
    c2_d = nc.dram_tensor("c2t", [64, 1], dt.float32, kind="ExternalInput")
    c3_d = nc.dram_tensor("c3t", [64, 1], dt.float32, kind="ExternalInput")
    bm_d = nc.dram_tensor("bmlp", [128, 20], dt.float32, kind="ExternalInput")
    b6_d = nc.dram_tensor("b6t", [12, 4], dt.float32, kind="ExternalInput")
    out_d = nc.dram_tensor("out", [12, 4, 256], dt.float32, kind="ExternalOutput")
    debug = bool(os.environ.get("NN_KERNEL_DEBUG"))
    if debug:
        dbgx2_d = nc.dram_tensor("dbg_x2", [128, 32, 10, 10], dt.bfloat16, kind="ExternalOutput")
        dbgx3_d = nc.dram_tensor("dbg_x3", [64, 256, 9, 9], dt.bfloat16, kind="ExternalOutput")
        dbgx4_d = nc.dram_tensor("dbg_x4", [128, 256, 25], dt.bfloat16, kind="ExternalOutput")
        dbgh1_d = nc.dram_tensor("dbg_h1", [128, 4, 256], dt.bfloat16, kind="ExternalOutput")
        dbgh5_d = nc.dram_tensor("dbg_h5", [128, 4, 256], dt.bfloat16, kind="ExternalOutput")

    with tile.TileContext(nc) as tc:
        with (
            tc.tile_pool(name="wts", bufs=1) as wpool,
            tc.tile_pool(name="x1", bufs=2) as x1pool,
            tc.tile_pool(name="x2", bufs=2) as x2pool,
            tc.tile_pool(name="big", bufs=1) as bigpool,
            tc.tile_pool(name="psc", bufs=4, space="PSUM") as psc,
            tc.tile_pool(name="psm", bufs=2, space="PSUM") as psm,
        ):
            # ---- resident weights/biases ----
            k1sb = wpool.tile([128, 128], dt.bfloat16)  # two copies: rows 0-47, 64-111
            nc.sync.dma_start(k1sb[0:48], k1_d.ap())
            nc.sync.dma_start(k1sb[64:112], k1_d.ap())
            k2sb = wpool.tile([128, 256], dt.bfloat16)
            nc.sync.dma_start(k2sb[:], k2_d.ap())
            k3sb = wpool.tile([64, 576], dt.bfloat16)
            nc.sync.dma_start(k3sb[:], k3_d.ap())
            c1sb = wpool.tile([128, 1], dt.float32)
            nc.sync.dma_start(c1sb[:], c1_d.ap())
            c2sb = wpool.tile([64, 1], dt.float32)
            nc.sync.dma_start(c2sb[:], c2_d.ap())
            c3sb = wpool.tile([64, 1], dt.float32)
            nc.sync.dma_start(c3sb[:], c3_d.ap())

            X3t = bigpool.tile([64, 256, 9, 9], dt.bfloat16)
            X4t = bigpool.tile([128, 256, 25], dt.bfloat16)
            Ha = bigpool.tile([128, 4, 256], dt.bfloat16)
            Hb = bigpool.tile([128, 4, 256], dt.bfloat16)
            OUTt = bigpool.tile([12, 4, 256], dt.float32)

            copy_ctr = 0

            def relu_copy(dst, src, bias):
                nonlocal copy_ctr
                copy_ctr += 1
                if copy_ctr % 2 == 0:
                    nc.scalar.activation(dst, src, AF.Relu, bias=bias)
                else:
                    nc.vector.tensor_scalar(dst, src, bias, 0.0, ALU.add, ALU.max)

            nc.vector.memset(X4t[64:128, :, 24:25], 0.0)

            # ---- conv1 + conv2, chunked over batch ----
            for c in range(8):
                X1t = x1pool.tile([128, HALF, 21, 21], dt.bfloat16)
                nc.sync.dma_start(
                    X1t[0:48], x1a_d.ap()[:, c * HALF : (c + 1) * HALF]
                )
                nc.sync.dma_start(
                    X1t[64:112], x1b_d.ap()[:, c * HALF : (c + 1) * HALF]
                )
                X2t = x2pool.tile([128, CHUNK, 10, 10], dt.bfloat16)

                # conv1: K=48 x 4 accumulation steps, 4 col-tiles (output parity)
                for g8 in range(8):  # alternate halves for LDW overlap
                    h, g = g8 % 2, g8 // 2
                    base = 64 * h
                    X1h = X1t[base : base + 48]
                    pt = psc.tile([128, 4, 10, 10], dt.float32, tag="conv")
                    for s in range(4):
                        a, a2 = s >> 1, s & 1
                        for q in range(4):
                            di, dj = q >> 1, q & 1
                            rhs = X1h[
                                :,
                                4 * g : 4 * g + 4,
                                (di + a) : (di + a + 19) : 2,
                                (dj + a2) : (dj + a2 + 19) : 2,
                            ]
                            nc.tensor.matmul(
                                pt[32 * q : 32 * q + 32],
                                k1sb[base : base + 48, 32 * s : 32 * s + 32],
                                rhs,
                                start=(s == 0),
                                stop=(s == 3),
                                tile_position=(base, 32 * q),
                            )
                    relu_copy(X2t[:, HALF * h + 4 * g : HALF * h + 4 * g + 4], pt[:], c1sb[:])

                if debug and c == 0:
                    nc.sync.dma_start(dbgx2_d.ap(), X2t[:])

                # conv2: K=128 x 4 steps
                for g in range(8):
                    pt2 = psc.tile([64, 4, 9, 9], dt.float32, tag="conv")
                    for s in range(4):
                        a, a2 = s >> 1, s & 1
                        rhs = X2t[:, 4 * g : 4 * g + 4, a : a + 9, a2 : a2 + 9]
                        nc.tensor.matmul(
                            pt2[:],
                            k2sb[:, 64 * s : 64 * s + 64],
                            rhs,
                            start=(s == 0),
                            stop=(s == 3),
                        )
                    b0 = c * CHUNK + 4 * g
                    relu_copy(X3t[:, b0 : b0 + 4], pt2[:], c2sb[:])

            # ---- conv3: K=64 x 9 steps ----
            for g in range(32):
                pt3 = psc.tile([64, 8, 7, 7], dt.float32, tag="conv")
                for k in range(9):
                    ky, kx = k // 3, k % 3
                    rhs = X3t[:, 8 * g : 8 * g + 8, ky : ky + 7, kx : kx + 7]
                    nc.tensor.matmul(
                        pt3[:],
                        k3sb[:, 64 * k : 64 * k + 64],
                        rhs,
                        start=(k == 0),
                        stop=(k == 8),
                    )
                ptf = pt3[:].rearrange("p b y x -> p b (y x)")
                relu_copy(X4t[0:64, 8 * g : 8 * g + 8, 0:25], ptf[:, :, 0:49:2], c3sb[:])
                relu_copy(X4t[64:128, 8 * g : 8 * g + 8, 0:24], ptf[:, :, 1:49:2], c3sb[:])

            if debug:
                nc.sync.dma_start(dbgx3_d.ap(), X3t[:])
                nc.sync.dma_start(dbgx4_d.ap(), X4t[:])

            # ---- MLP weights (loaded during conv phase; emitted here) ----
            w1sb = wpool.tile([128, 4 * 25 * 128], dt.bfloat16)
            nc.sync.dma_start(w1sb[:], w1_d.ap())
            w25sb = wpool.tile([128, 16 * 128], dt.bfloat16)
            nc.sync.dma_start(w25sb[:], w25_d.ap())
            w6sb = wpool.tile([128, 48], dt.bfloat16)
            nc.sync.dma_start(w6sb[:], w6_d.ap())
            bmsb = wpool.tile([128, 20], dt.float32)
            nc.sync.dma_start(bmsb[:], bm_d.ap())
            b6sb = wpool.tile([12, 4], dt.float32)
            nc.sync.dma_start(b6sb[:], b6_d.ap())

            # ---- MLP layer 1: all 8 experts, expert-paired M=128, K=128 ----
            for p in range(4):
                ptm = psm.tile([128, 256], dt.float32, tag="mlp")
                for t in range(25):
                    nc.tensor.matmul(
                        ptm[:],
                        w1sb[:, (p * 25 + t) * 128 : (p * 25 + t) * 128 + 128],
                        X4t[:, :, t],
                        start=(t == 0),
                        stop=(t == 24),
                    )
                relu_copy(Ha[:, p, :], ptm[:], bmsb[:, p : p + 1])

            if debug:
                nc.sync.dma_start(dbgh1_d.ap(), Ha[:])

            # ---- MLP layers 2-5: block-diagonal expert pairs ----
            src, dst = Ha, Hb
            for l in range(4):
                for p in range(4):
                    idx = l * 4 + p
                    ptm = psm.tile([128, 256], dt.float32, tag="mlp")
                    nc.tensor.matmul(
                        ptm[:],
                        w25sb[:, 128 * idx : 128 * idx + 128],
                        src[:, p, :],
                        start=True,
                        stop=True,
                    )
                    relu_copy(dst[:, p, :], ptm[:], bmsb[:, (l + 1) * 4 + p : (l + 1) * 4 + p + 1])
                src, dst = dst, src

            if debug:
                nc.sync.dma_start(dbgh5_d.ap(), src[:])

            # ---- MLP layer 6 (no relu) ----
            for p in range(4):
                pt6 = psm.tile([12, 256], dt.float32, tag="mlp")
                nc.tensor.matmul(
                    pt6[:],
                    w6sb[:, 12 * p : 12 * p + 12],
                    src[:, p, :],
                    start=True,
                    stop=True,
                )
                nc.vector.tensor_scalar(
                    OUTt[:, p, :], pt6[:], b6sb[:, p : p + 1], None, ALU.add
                )

            nc.sync.dma_start(out_d.ap(), OUTt[:])

    nc.compile()
    return nc


_prog_cache = {}
LAST_RESULTS = None


def kernel(**inputs):
    state = np.asarray(inputs["state"], dtype=np.float32)
    rm_state = np.asarray(inputs["rm_state"]).astype(np.int64)
    k1 = np.asarray(inputs["k1"], dtype=np.float32)
    c1 = np.asarray(inputs["c1"], dtype=np.float32)
    k2 = np.asarray(inputs["k2"], dtype=np.float32)
    c2 = np.asarray(inputs["c2"], dtype=np.float32)
    k3 = np.asarray(inputs["k3"], dtype=np.float32)
    c3 = np.asarray(inputs["c3"], dtype=np.float32)
    Ws = [np.asarray(inputs[f"W{i}"], dtype=np.float32) for i in range(1, 7)]
    Bs = [np.asarray(inputs[f"B{i}"], dtype=np.float32) for i in range(1, 7)]

    # ---- host prep ----
    # space-to-depth: [core, b, (i,dy), (j,dx), c] -> [core, (dy,dx,c), b, i, j]
    v = state.reshape(NCORES, BC, 21, 4, 21, 4, 3)
    s2d = np.ascontiguousarray(np.transpose(v, (0, 3, 5, 6, 1, 2, 4))).reshape(
        NCORES, 48, BC, 21, 21
    )
    # chunk c processes samples [32c, 32c+32): first 16 via x1a, next 16 via
    # x1b (the two SBUF partition-halves), so device order == global order.
    vv = s2d.reshape(NCORES, 48, 8, 2, 16, 21, 21)
    x1a = np.ascontiguousarray(vv[:, :, :, 0]).reshape(NCORES, 48, 128, 21, 21).astype(BF16)
    x1b = np.ascontiguousarray(vv[:, :, :, 1]).reshape(NCORES, 48, 128, 21, 21).astype(BF16)

    # conv weights: partition=(dy,dx,ci), free=(step, co)
    k1r = (
        k1.reshape(32, 3, 2, 4, 2, 4)
        .transpose(2, 4, 3, 5, 1, 0)
        .reshape(4, 48, 32)
        .transpose(1, 0, 2)
        .reshape(48, 128)
        .astype(BF16)
    )
    k2r = (
        k2.reshape(64, 32, 2, 2, 2, 2)
        .transpose(2, 4, 3, 5, 1, 0)
        .reshape(4, 128, 64)
        .transpose(1, 0, 2)
        .reshape(128, 256)
        .astype(BF16)
    )
    k3r = (
        k3.transpose(2, 3, 1, 0).reshape(9, 64, 64).transpose(1, 0, 2).reshape(64, 576)
    ).astype(BF16)

    # W1: [E, 3136, 64] -> s-pair tiles [partition=(spar,c), (pair, t, e2*64+m)]
    w1v = np.ascontiguousarray(
        Ws[0].reshape(4, 2, 64, 49, 64).transpose(0, 3, 2, 1, 4)
    ).reshape(4, 49, 64, 128)
    w1r = np.zeros((4, 25, 128, 128), np.float32)
    w1r[:, :, 0:64] = w1v[:, 0::2]
    w1r[:, :24, 64:128] = w1v[:, 1::2]
    # -> [128, 4*25*128] partition-major
    w1r = np.ascontiguousarray(w1r.transpose(2, 0, 1, 3)).reshape(128, 4 * 25 * 128).astype(BF16)

    # W2..W5 block-diagonal expert pairs: [128, 16*128] (col = (l*4+p)*128 + m')
    w25 = np.zeros((128, 16, 128), np.float32)
    for l in range(4):
        Wl = Ws[1 + l]
        for p in range(4):
            for e2 in range(2):
                w25[64 * e2 : 64 * e2 + 64, l * 4 + p, 64 * e2 : 64 * e2 + 64] = Wl[
                    2 * p + e2
                ]
    w25 = w25.reshape(128, 16 * 128).astype(BF16)

    w6 = np.zeros((128, 4, 12), np.float32)
    for p in range(4):
        for e2 in range(2):
            w6[64 * e2 : 64 * e2 + 64, p, 6 * e2 : 6 * e2 + 6] = Ws[5][2 * p + e2]
    w6 = w6.reshape(128, 48).astype(BF16)

    c1t = np.tile(c1, 4)[:, None].astype(np.float32)
    c2t = c2[:, None].astype(np.float32)
    c3t = c3[:, None].astype(np.float32)
    bmlp = np.zeros((128, 20), np.float32)
    for l in range(5):
        Bl = Bs[l]
        for p in range(4):
            bmlp[:, l * 4 + p] = np.concatenate([Bl[2 * p], Bl[2 * p + 1]])
    b6t = np.zeros((12, 4), np.float32)
    for p in range(4):
        b6t[:, p] = np.concatenate([Bs[5][2 * p], Bs[5][2 * p + 1]])

    # ---- build + run ----
    trace = bool(os.environ.get("NN_KERNEL_TRACE"))
    _install_axon_prof_shim()
    if "nc" not in _prog_cache:
        _prog_cache["nc"] = _build_program()
    nc = _prog_cache["nc"]

    shared = {
        "k1r": k1r,
        "k2r": k2r,
        "k3r": k3r,
        "w1r": w1r,
        "w25r": w25,
        "w6r": w6,
        "c1t": c1t,
        "c2t": c2t,
        "c3t": c3t,
        "bmlp": bmlp,
        "b6t": b6t,
    }
    in_maps = [
        {"x1a": np.ascontiguousarray(x1a[c]), "x1b": np.ascontiguousarray(x1b[c]), **shared}
        for c in range(NCORES)
    ]

    from concourse.bass_utils import run_bass_kernel_spmd

    res = run_bass_kernel_spmd(
        nc, in_maps, core_ids=list(range(NCORES)), trace=trace
    )
    if trace and res.exec_time_ns is not None:
        print(f"HW exec time: {res.exec_time_ns} ns")

    global LAST_RESULTS
    LAST_RESULTS = res.results

    # ---- host gather ----
    outs = []
    for c in range(NCORES):
        r = res.results[c]["out"].reshape(2, 6, 4, 256)
        outs.append(r.transpose(2, 0, 3, 1).reshape(8, 256, 6))
    full = np.stack(outs)  # [core, e, b, a]
    per_sample = full.transpose(0, 2, 1, 3).reshape(B, E, A)
    return per_sample[np.arange(B), rm_state].astype(np.float32)
